# revision 1
# baseline (speedup 1.0000x reference)
# Area-attention (AAttn) kernel for Trainium2, SPMD over 8 NeuronCores.
#
# Reference computation (B=4, C=256, H=W=64, 8 heads, head_dim=32, AREA=4):
#   qk = conv1x1(x, Wqk) + bqk ; v4 = conv1x1(x, Wv) + bv
#   pp = dwconv5x5(v4, Wpe) + bpe          (depthwise, zero pad 2)
#   area split: 16 chunks of 1024 tokens (16 image rows each)
#   attn = softmax(q k^T / sqrt(32)) v     per (chunk, head)
#   out  = conv1x1(attn_out + pp, Wproj) + bproj
#
# Sharding: data-parallel over the 16 (B*area) chunks -> 2 chunks per core.
#
# v2 design notes (PE issues ~1 moving column/cycle total, so minimize
# total moving columns; ScalarE exp is the secondary floor):
#   - PV uses fused [V|ones] M=33 stationaries: the softmax denominator
#     rides along as psum row 32/64 instead of a second full matmul.
#   - token-major V (PV stationary) built with PE transposes of v4
#     instead of 96 projection matmuls.
#   - dwconv runs on the DVE as 50 shifted-window fp16 multiply-accumulate
#     taps, freeing ~100k PE columns.
#   - denominators DMA'd to sbuf lanes 0/1, one reciprocal, gpsimd
#     partition-broadcast, then norm multiplies.
#   - PSUM: "big" pool (qk chains / score tiles / proj) 2x[128,1024]f32
#     + "pv" pool 2x[128,1024]f32 = 8 banks exactly.

import numpy as np
import ml_dtypes

import concourse.bass as bass
import concourse.bacc as bacc
import concourse.mybir as mybir
import concourse.tile as tile
from concourse.bass_utils import run_bass_kernel_spmd
from concourse.masks import make_identity

BF16 = mybir.dt.bfloat16
F16 = mybir.dt.float16
F32 = mybir.dt.float32
NPBF = ml_dtypes.bfloat16
ALU = mybir.AluOpType
ACT = mybir.ActivationFunctionType

TRACE = False
LAST_EXEC_NS = None
LAST_RESULTS = None

EXTR, EXTC = 36, 68
NEXT = EXTR * EXTC        # 2448 ext tokens
NTOK = 2048               # 32x64 center tokens per core
SCALE = float(1.0 / np.sqrt(32.0))


_cached_nc = None


def _build():
    nc = bacc.Bacc()

    x_d = nc.declare_dram_parameter("x", [128, 2, NEXT], BF16, isOutput=False)
    vmask_d = nc.declare_dram_parameter("vmask", [1, NEXT], BF16, isOutput=False)
    wqk_d = nc.declare_dram_parameter("wqk", [128, 2, 512], BF16, isOutput=False)
    wv_d = nc.declare_dram_parameter("wv", [128, 2, 256], BF16, isOutput=False)
    wproj_d = nc.declare_dram_parameter("wproj", [128, 2, 256], BF16, isOutput=False)
    bqk_d = nc.declare_dram_parameter("bqk", [128, 4], F32, isOutput=False)
    bvrow_d = nc.declare_dram_parameter("bvrow", [1, 256], BF16, isOutput=False)
    bproj_d = nc.declare_dram_parameter("bproj", [128, 2], F32, isOutput=False)
    wpe_d = nc.declare_dram_parameter("wpe", [128, 2, 25], F32, isOutput=False)
    bpe_d = nc.declare_dram_parameter("bpe", [128, 2], F32, isOutput=False)
    out_d = nc.declare_dram_parameter("out", [2, 128, NTOK], F32, isOutput=True)

    with tile.TileContext(nc) as tc:
        with (
            tc.tile_pool(name="singles", bufs=1) as singles,
            tc.tile_pool(name="ptp", bufs=2) as ptp,
            tc.tile_pool(name="bigp", bufs=2, space="PSUM") as bigp,
            tc.tile_pool(name="pvp", bufs=2, space="PSUM") as pvp,
            tc.tile_pool(name="rr", bufs=2) as rrp,
            tc.tile_pool(name="rb", bufs=2) as rbp,
            tc.tile_pool(name="onp", bufs=2) as onp,
            tc.tile_pool(name="ys", bufs=2) as ys,
        ):
            # ---- load everything to SBUF ----
            x_sb = singles.tile([128, 2, NEXT], BF16, tag="x")
            nc.sync.dma_start(out=x_sb, in_=x_d[:, :, :])
            wqk_sb = singles.tile([128, 2, 512], BF16, tag="wqk")
            nc.sync.dma_start(out=wqk_sb, in_=wqk_d[:, :, :])
            wv_sb = singles.tile([128, 2, 256], BF16, tag="wv")
            nc.sync.dma_start(out=wv_sb, in_=wv_d[:, :, :])
            wproj_sb = singles.tile([128, 2, 256], BF16, tag="wproj")
            nc.sync.dma_start(out=wproj_sb, in_=wproj_d[:, :, :])
            bqk_sb = singles.tile([128, 4], F32, tag="bqk")
            nc.sync.dma_start(out=bqk_sb, in_=bqk_d[:, :])
            bvrow_sb = singles.tile([1, 256], BF16, tag="bvrow")
            nc.sync.dma_start(out=bvrow_sb, in_=bvrow_d[:, :])
            bproj_sb = singles.tile([128, 2], F32, tag="bproj")
            nc.sync.dma_start(out=bproj_sb, in_=bproj_d[:, :])
            wpe_sb = singles.tile([128, 2, 25], F32, tag="wpe")
            nc.sync.dma_start(out=wpe_sb, in_=wpe_d[:, :, :])
            bpe_sb = singles.tile([128, 2], F32, tag="bpe")
            nc.sync.dma_start(out=bpe_sb, in_=bpe_d[:, :])
            vmask_sb = singles.tile([1, NEXT], BF16, tag="vmask")
            nc.sync.dma_start(out=vmask_sb, in_=vmask_d[:, :])

            ident = singles.tile([128, 128], BF16, tag="ident")
            make_identity(nc, ident)

            # DVE touches: absorb DMA-queue deps so TensorScalar ops (1-wait
            # HW limit) only carry their producer-engine wait.
            scr = singles.tile([128, 64], F32, tag="scr")
            nc.vector.tensor_copy(out=scr[:, 0:4], in_=bqk_sb[:, :])
            nc.vector.tensor_copy(out=scr[:, 4:6], in_=bproj_sb[:, :])
            nc.vector.tensor_copy(out=scr[:, 6:8], in_=bpe_sb[:, :])
            nc.vector.tensor_copy(out=scr[:, 8:58], in_=wpe_sb.rearrange("p a b -> p (a b)"))

            q_sb = singles.tile([128, 2, NTOK], BF16, tag="q")
            k_sb = singles.tile([128, 2, NTOK], BF16, tag="k")
            v4_sb = singles.tile([128, 2, NEXT], BF16, tag="v4")
            # token-major V with fused ones col per head: [V_h(32) | 1]
            va_sb = singles.tile([128, 16, 264], BF16, tag="va")
            nc.vector.memset(va_sb[:, :, :], 1.0)
            outn_sb = singles.tile([128, 2, NTOK], BF16, tag="outn")
            pp_sb = singles.tile([128, 2, NTOK], F16, tag="pp")

            x_v = x_sb.rearrange("p k (r w) -> p k r w", r=EXTR)
            v4_v = v4_sb.rearrange("p o (r w) -> p o r w", r=EXTR)
            pp_v = pp_sb.rearrange("p o (r w) -> p o r w", r=32)

            # ---- P1: projection emitters (most run as pipeline fillers) ----
            def emit_qk(o, half):
                ps = bigp.tile([128, 1024], F32, tag="big", name="qkps")
                for ch in range(2):
                    sl = slice(ch * 512, (ch + 1) * 512)
                    r0 = 2 + 16 * half + 8 * ch
                    for kt in range(2):
                        nc.tensor.matmul(
                            ps[:, sl],
                            wqk_sb[:, kt, o * 128:(o + 1) * 128],
                            x_v[:, kt, r0:r0 + 8, 2:66],
                            start=(kt == 0), stop=(kt == 1),
                        )
                dst = q_sb if o < 2 else k_sb
                nc.vector.tensor_scalar(
                    out=dst[:, o % 2, half * 1024:(half + 1) * 1024], in0=ps[:, :],
                    scalar1=bqk_sb[:, o:o + 1], scalar2=None, op0=ALU.add,
                )

            def emit_v4(o, half):
                n0 = half * 1024
                n1 = min(n0 + 1024, NEXT)
                ps = bigp.tile([128, 1024], F32, tag="big", name="v4ps")
                for ch in range((n1 - n0 + 511) // 512):
                    sl = slice(n0 + ch * 512, min(n0 + (ch + 1) * 512, n1))
                    psl = slice(ch * 512, ch * 512 + (sl.stop - sl.start))
                    for kt in range(2):
                        nc.tensor.matmul(
                            ps[:, psl], wv_sb[:, kt, o * 128:(o + 1) * 128],
                            x_sb[:, kt, sl], start=(kt == 0), stop=False,
                        )
                    nc.tensor.matmul(
                        ps[:, psl], bvrow_sb[:, o * 128:(o + 1) * 128],
                        vmask_sb[:, sl], start=False, stop=True,
                    )
                nc.vector.tensor_copy(out=v4_sb[:, o, n0:n1], in_=ps[:, 0:n1 - n0])

            # token-major V via PE transpose; va cols h*33..h*33+31 get V_h,
            # col h*33+32 stays the memset 1.0 (the fused denominator col).
            def emit_vtr(blk, kt):
                ps = pvp.tile([128, 1024], F32, tag="pv", name="vtr")
                dst = ps[:, 0:64].bitcast(BF16)
                for rr in range(2):
                    nc.tensor.transpose(
                        dst[64 * rr:64 * rr + 64, :],
                        v4_v[:, kt, 2 + 2 * blk + rr, 2:66], ident,
                        tile_position=(0, 64 * rr))
                # psum [128tok, 4 heads x 32] -> va cols (4kt+j)*33 ..+31
                # (col h*33+32 keeps the memset 1.0: the fused denom col)
                src = dst.rearrange("p (h d) -> p h d", h=4)
                dv = va_sb[:, blk, 33 * 4 * kt:33 * 4 * kt + 132]
                dv = dv.rearrange("p (h d) -> p h d", d=33)[:, :, 0:32]
                nc.vector.tensor_copy(out=dv, in_=src)

            # ---- dwconv on DVE: fp16 acc, 25 shifted-window taps per half --
            def emit_dw(o, t):
                dy, dx = t // 5, t % 5
                src = v4_v[:, o, dy:dy + 32, dx:dx + 64]
                dst = pp_v[:, o, :, :]
                if t == 0:
                    nc.vector.tensor_scalar_mul(
                        out=dst, in0=src, scalar1=wpe_sb[:, o, 0:1])
                else:
                    nc.vector.scalar_tensor_tensor(
                        out=dst, in0=src, scalar=wpe_sb[:, o, t:t + 1],
                        in1=dst, op0=ALU.mult, op1=ALU.add)

            # ---- PV filler for a finished group (c, hg2): one (qc, kt) ----
            # Both hl chains accumulate at tile_position (0,0) into disjoint
            # 512-col strips of one psum tile (PE col strips >=64 corrupt
            # when interleaved with the score matmuls). The [1|V] stationary
            # leaves each denominator at psum ROW 0 (V on rows 1-32); one
            # psum-direct reciprocal at p0 covers both heads, a DMA with a
            # free-dim-broadcast source replicates it to lanes 1-32, one
            # fused multiply normalizes both heads, and DMA partition
            # shifts land them in outn.
            def emit_pv(c, hg2, pt, qc, kt, pvt):
                o, s0 = hg2 // 2, 2 * (hg2 % 2)
                if kt == 0:
                    pvt[qc] = pvp.tile([128, 1024], F32, tag="pv", name="pv")
                pv = pvt[qc]
                for hl in range(2):
                    h = 4 * o + s0 + hl
                    nc.tensor.matmul(
                        pv[0:33, hl * 512:hl * 512 + 512],
                        va_sb[:, c * 8 + kt, 33 * h:33 * h + 33],
                        pt[:, kt, qc, hl * 512:hl * 512 + 512],
                        start=(kt == 0), stop=(kt == 7),
                        tile_position=(0, 0),
                        skip_group_check=True,
                    )
                if kt != 7:
                    return
                # den row (psum p32) -> sbuf, DMA-replicate to lanes 0-31,
                # reciprocal there (custom DVE ops only work at p0), then
                # one fused multiply normalizes both heads.
                dd = rrp.tile([64, 1024], F32, tag="dd", name="dd")
                nc.vector.tensor_copy(out=dd[32:33, :], in_=pv[32:33, :])
                rb = rbp.tile([32, 1024], F32, tag="rb", name="rb")
                nc.sync.dma_start(
                    out=rb[0:32, :],
                    in_=dd[32:33, :].rearrange("p (u f) -> p u f", u=1)
                    .broadcast_to([1, 32, 1024]))
                rbr = rbp.tile([32, 1024], F32, tag="rbr", name="rbr")
                nc.vector.reciprocal_approx_fast(out=rbr[:, :], in_=rb[:, :])
                on = onp.tile([32, 1024], BF16, tag="on", name="on")
                nc.vector.tensor_mul(
                    out=on[:, :], in0=pv[0:32, :], in1=rbr[:, :])
                tok = slice(c * 1024 + qc * 512, c * 1024 + qc * 512 + 512)
                r0, r1 = 32 * s0, 32 * (s0 + 1)
                nc.sync.dma_start(
                    out=outn_sb[r0:r0 + 32, o, tok], in_=on[:, 0:512])
                nc.sync.dma_start(
                    out=outn_sb[r1:r1 + 32, o, tok], in_=on[:, 512:1024])

            # ---- tail emitters: outn += pp + bpe, then proj + bias + DMA ---
            def emit_merge(o, ch):
                sl = slice(ch * 512, (ch + 1) * 512)
                nc.vector.scalar_tensor_tensor(
                    out=outn_sb[:, o, sl],
                    in0=pp_sb[:, o, sl], scalar=bpe_sb[:, o:o + 1],
                    in1=outn_sb[:, o, sl],
                    op0=ALU.add, op1=ALU.add,
                )

            def emit_proj(o, half):
                ps = bigp.tile([128, 1024], F32, tag="big", name="projps")
                for ch in range(2):
                    for kt in range(2):
                        nc.tensor.matmul(
                            ps[:, ch * 512:(ch + 1) * 512],
                            wproj_sb[:, kt, o * 128:(o + 1) * 128],
                            outn_sb[:, kt, half * 1024 + ch * 512:
                                    half * 1024 + (ch + 1) * 512],
                            start=(kt == 0), stop=(kt == 1),
                        )
                y_sb = ys.tile([128, 1024], F32, tag="y")
                nc.vector.tensor_scalar(
                    out=y_sb[:, :], in0=ps[:, :],
                    scalar1=bproj_sb[:, o:o + 1], scalar2=None, op0=ALU.add,
                )
                nc.sync.dma_start(
                    out=out_d[o, :, half * 1024:(half + 1) * 1024], in_=y_sb[:, :])

            # ---- schedule ----
            for o in (0, 2):
                for half in range(2):
                    emit_qk(o, half)

            from collections import deque

            def emit_vtr4(i4):
                for j in range(4):
                    blk, kt = (i4 * 4 + j) // 2, (i4 * 4 + j) % 2
                    emit_vtr(blk, kt)

            extra = deque()
            for o, half in ((1, 0), (1, 1), (3, 0), (3, 1)):
                extra.append(lambda o=o, h=half: emit_qk(o, h))
            for half in range(3):
                for o in range(2):
                    extra.append(lambda o=o, h=half: emit_v4(o, h))
            for i4 in range(8):
                extra.append(lambda i4=i4: emit_vtr4(i4))
            for t in range(25):
                for o in range(2):
                    extra.append(lambda o=o, t=t: emit_dw(o, t))
            # tail1: c=0 token merges + proj half0 (ready once groups (0,*)
            # are normalized, i.e. from gi>=5); tail2 after the last PV.
            tail = deque()
            for ch in (0, 1):
                for o in range(2):
                    tail.append(lambda o=o, c=ch: emit_merge(o, c))
            for o in range(2):
                tail.append(lambda o=o: emit_proj(o, 0))
            tail2 = deque()
            for ch in (2, 3):
                for o in range(2):
                    tail2.append(lambda o=o, c=ch: emit_merge(o, c))
            for o in range(2):
                tail2.append(lambda o=o: emit_proj(o, 1))

            groups = [(c, hg2) for c in range(2) for hg2 in range(4)]
            # pvq carries PV fillers with a permanent 2-slot lag so the
            # first PV of a group never waits on a just-issued exp.
            pvq = deque([None, None])

            def queue_pv(c, hg2, pt):
                pvt = {}
                for qc in range(2):
                    for pkt in range(8):
                        pvq.append(
                            lambda qc=qc, pkt=pkt, c=c, h=hg2, p=pt, pvt=pvt:
                            emit_pv(c, h, p, qc, pkt, pvt))

            prev = None
            for gi, (c, hg2) in enumerate(groups):
                o, s0 = hg2 // 2, 2 * (hg2 % 2)
                pt = ptp.tile([128, 8, 2, 1024], BF16, tag="pt")
                if prev is not None:
                    queue_pv(*prev)
                for kt in range(8):
                    for qc in range(2):
                        sc = bigp.tile([128, 1024], F32, tag="big", name="sc")
                        ktok = c * 1024 + kt * 128
                        qtok = c * 1024 + qc * 512
                        for i in range(2):
                            nc.tensor.matmul(
                                sc[:, i * 512:(i + 1) * 512],
                                k_sb[32 * (s0 + i):32 * (s0 + i) + 32, o,
                                     ktok:ktok + 128],
                                q_sb[32 * (s0 + i):32 * (s0 + i) + 32, o,
                                     qtok:qtok + 512],
                                start=True, stop=True,
                                tile_position=(32 * (s0 + i), 0),
                            )
                        nc.scalar.activation(
                            out=pt[:, kt, qc, :], in_=sc[:, :],
                            func=ACT.Exp, scale=SCALE,
                        )
                        if pvq:
                            f = pvq.popleft()
                            if f is not None:
                                f()
                        if extra:
                            extra.popleft()()
                        if gi >= 5 and tail and qc == 1 and kt % 3 == 2:
                            tail.popleft()()
                prev = (c, hg2, pt)
            while extra:
                extra.popleft()()
            while pvq:
                f = pvq.popleft()
                if f is not None:
                    f()
            while tail:
                tail.popleft()()
            queue_pv(*prev)
            while pvq:
                pvq.popleft()()
            while tail2:
                tail2.popleft()()

    nc.compile()
    return nc


def _shards(x, Wqk, bqk, Wv, bv, Wpe, bpe, Wproj, bproj):
    B, C, H, W = x.shape
    wqk = np.ascontiguousarray(
        Wqk.T.reshape(2, 128, 512).transpose(1, 0, 2)).astype(NPBF)
    wv = np.ascontiguousarray(
        Wv.T.reshape(2, 128, 256).transpose(1, 0, 2)).astype(NPBF)
    wproj = np.ascontiguousarray(
        Wproj.T.reshape(2, 128, 256).transpose(1, 0, 2)).astype(NPBF)
    bqks = np.ascontiguousarray(bqk.reshape(4, 128).T).astype(np.float32)
    bvrow = bv.reshape(1, 256).astype(NPBF)
    bprojs = np.ascontiguousarray(bproj.reshape(2, 128).T).astype(np.float32)
    wpe = np.ascontiguousarray(
        Wpe.reshape(256, 25).reshape(2, 128, 25).transpose(1, 0, 2)
    ).astype(np.float32)
    bpes = np.ascontiguousarray(bpe.reshape(2, 128).T).astype(np.float32)

    common = dict(wqk=wqk, wv=wv, wproj=wproj, bqk=bqks, bvrow=bvrow,
                  bproj=bprojs, wpe=wpe, bpe=bpes)

    in_maps = []
    for core in range(8):
        b, half = core // 2, core % 2
        xe = np.zeros((256, EXTR, EXTC), np.float32)
        r0 = half * 32
        lo, hi = max(r0 - 2, 0), min(r0 + 34, 64)
        xe[:, (lo - (r0 - 2)):(hi - (r0 - 2)), 2:66] = x[b, :, lo:hi, :]
        xs = np.ascontiguousarray(
            xe.reshape(2, 128, NEXT).transpose(1, 0, 2)).astype(NPBF)
        vm = np.zeros((EXTR, EXTC), np.float32)
        vm[(lo - (r0 - 2)):(hi - (r0 - 2)), 2:66] = 1.0
        vm = vm.reshape(1, NEXT).astype(NPBF)
        in_maps.append(dict(common, x=xs, vmask=vm))
    return in_maps


def kernel(**inputs):
    global _cached_nc, LAST_EXEC_NS, LAST_RESULTS
    if _cached_nc is None:
        _cached_nc = _build()
    inputs = {k: np.asarray(v) for k, v in inputs.items()}
    in_maps = _shards(**inputs)
    res = run_bass_kernel_spmd(_cached_nc, in_maps, list(range(8)), trace=TRACE)
    LAST_EXEC_NS = res.exec_time_ns
    LAST_RESULTS = res
    out = np.zeros((4, 256, 64, 64), np.float32)
    for core in range(8):
        b, half = core // 2, core % 2
        y = np.asarray(res.results[core]["out"]).reshape(256, 32, 64)
        out[b, :, half * 32:(half + 1) * 32, :] = y
    return out

